# revision 17
# baseline (speedup 1.0000x reference)
"""Trainium2 Bass kernel for fused multi-head attention (dense transformer block).

Problem: y = proj(softmax(QK^T/sqrt(d)) V) for x [4, 2048, 512], 16 heads, d=32.

Sharding (8 cores): core c handles batch b = c//2 and head-group hg = c%2
(8 heads each). Inside a core everything is computed transposed
(feature-major) so that softmax denominators and the output projection
need no on-chip transposes:

  x^T  [c, t]        via PE transpose of x tiles
  Q^T/K^T [d, t]     = W^T x^T   (lhsT = W slices, natural layout)
  V    [t, d]        = x W_v     (lhsT = x^T tiles)
  S^T  [k, q]        = K^T.T @ Q^T  per head (contraction d=32, f32r)
  P^T  = exp(S^T / sqrt(d))      (ScalarE, fused scale; scores are small so
                                  no max-subtraction is needed: |s| < ~2)
  [sums | 0 | O^T] = [1 | 0 | V].T @ P^T  per head (M=64 matmul; row 0 =
                                  softmax denominators, rows 32:64 = O^T,
                                  accumulated over k-tiles in one PSUM bank)
  O^T /= sums                    (reciprocal row 0 + partition broadcast + mul)
  out^T [c_out, t]   = W_p.T @ O^T  (lhsT = w_proj rows, natural layout)

V-bias and output bias are folded on the host into a single vector added
after the cross-head-group reduction (softmax rows sum to 1, so the V bias
contributes exactly b_v @ w_proj to every token).

PSUM budget (8 banks): sT tag [128, 2, 512] x 2 bufs = 4 banks; os0..os3
tags [128, 512] x 1 buf = 4 banks (shared by QKV/transpose/projection
evictions and the per-head O/sums accumulators).
"""

import sys

sys.path.insert(0, "/opt/trn_rl_repo")

import numpy as np

N_CORES = 8
B, T, C = 4, 2048, 512
N_HEADS, HEAD_DIM = 16, 32
HPC = 8           # heads per core
NG = 2            # head groups of 4 per core
SCALE = 1.0 / np.sqrt(np.float32(HEAD_DIM))
CT = C // 128     # 4 c-tiles
TT = T // 128     # 16 t-tiles
QC = T // 512     # 4 q-chunks
KT = T // 128     # 16 k-tiles

_CACHE = {}


def _build(stages=("prep", "attn", "proj")):
    import concourse.bass as bass
    import concourse.tile as tile
    from concourse import bacc, mybir
    from concourse.masks import make_identity

    f32 = mybir.dt.float32
    f32r = mybir.dt.float32r
    Exp = mybir.ActivationFunctionType.Exp
    ts = bass.ts

    nc = bacc.Bacc("TRN2", target_bir_lowering=False, debug=False,
                   num_devices=N_CORES)

    xb_d = nc.dram_tensor("xb", (T, C), f32, kind="ExternalInput")
    wq_d = nc.dram_tensor("wq", (C, 256), f32, kind="ExternalInput")
    wk_d = nc.dram_tensor("wk", (C, 256), f32, kind="ExternalInput")
    wv_d = nc.dram_tensor("wv", (C, 256), f32, kind="ExternalInput")
    bq_d = nc.dram_tensor("bq", (2, 128, 1), f32, kind="ExternalInput")
    bk_d = nc.dram_tensor("bk", (2, 128, 1), f32, kind="ExternalInput")
    wp_d = nc.dram_tensor("wp", (256, C), f32, kind="ExternalInput")
    outT_d = nc.dram_tensor("outT", (C, T), f32, kind="ExternalOutput")
    oT_d = nc.dram_tensor("oTd", (2, 128, T), mybir.dt.float32r,
                          kind="Internal")

    from contextlib import ExitStack

    with tile.TileContext(nc) as tc, ExitStack() as ctx:
        persist = ctx.enter_context(tc.tile_pool(name="persist", bufs=1))
        psum = ctx.enter_context(tc.tile_pool(name="psum", bufs=1,
                                              space="PSUM"))
        misc = ctx.enter_context(tc.tile_pool(name="misc", bufs=2))
        pTp = ctx.enter_context(tc.tile_pool(name="pTp", bufs=2))
        hstage = ctx.enter_context(tc.tile_pool(name="hstage", bufs=1))
        qstage = ctx.enter_context(tc.tile_pool(name="qstage", bufs=1))
        ostage = ctx.enter_context(tc.tile_pool(name="ostage", bufs=2))

        def os_tile(i, name):
            return psum.tile([128, 512], f32, tag=f"os{i}", name=name)

        # ---- persistent SBUF ----
        identity = persist.tile([128, 128], f32)
        make_identity(nc, identity[:])
        wp = persist.tile([128, 2, C], f32r)
        bq = persist.tile([128, 2], f32)
        bk = persist.tile([128, 2], f32)
        qT = persist.tile([128, 2, T], f32r)   # [d-in-tile, g, t]
        kT = persist.tile([128, 2, T], f32r)
        # vx: [t-in-tile, t-tile, core-head, 64]:
        #   col 0 = ones, cols 1:32 = zeros, cols 32:64 = V_h
        vx = persist.tile([128, TT, HPC, 64], f32r)
        ones128 = persist.tile([128, 128], f32)
        nc.vector.memset(ones128[:], 1.0)
        nc.vector.tensor_copy(
            vx[:, :, :, 0:1],
            ones128[:].rearrange("p (a b one) -> p a b one", a=TT, b=HPC,
                                 one=1),
        )
        zero31 = persist.tile([128, HPC, 31], f32)
        nc.vector.memset(zero31[:], 0.0)

        k_st = [hstage.tile([32, T], f32r, tag=f"k_st{h}",
                            name=f"k_st{h}") for h in range(4)]

        def emit_weights(wq, wk, wv):
            with tc.tile_pool(name="wstage", bufs=3) as wstage:
                for kc in range(CT):
                    for wdst, wsrc in ((wq, wq_d), (wk, wk_d), (wv, wv_d)):
                        wst = wstage.tile([128, 256], f32, tag="wst",
                                          name="wst")
                        nc.scalar.dma_start(wst[:], wsrc.ap()[ts(kc, 128), :])
                        nc.vector.tensor_copy(wdst[:, kc, :], wst[:])
                for yt in range(2):
                    for half in range(2):
                        wst4 = wstage.tile([128, 256], f32, tag="wst",
                                           name="wst4")
                        nc.scalar.dma_start(
                            wst4[:], wp_d.ap()[ts(yt, 128), ts(half, 256)])
                        nc.vector.tensor_copy(wp[:, yt, ts(half, 256)],
                                              wst4[:])
                for m in range(2):
                    nc.sync.dma_start(bq[:, m:m + 1], bq_d.ap()[m])
                    nc.sync.dma_start(bk[:, m:m + 1], bk_d.ap()[m])

        def emit_transposes(xT, xstage, tch):
            # x^T via PE transposes: xT [c-in-tile, c-tile, t]
            for tt in range(4 * tch, 4 * tch + 4):
                xst = xstage.tile([128, 512], f32, tag="xst", name="xst")
                eng = nc.sync if tt % 2 == 0 else nc.scalar
                eng.dma_start(xst[:], xb_d.ap()[ts(tt, 128), :])
                for kc in range(CT):
                    tps = os_tile(kc, "tps")
                    nc.tensor.transpose(tps[:, 0:128], xst[:, ts(kc, 128)],
                                        identity[:])
                    nc.vector.tensor_copy(xT[:, kc, ts(tt, 128)],
                                          tps[:, 0:128])

        def emit_qk(xT, wq, wk, g, only_tch=None):
            for tch in (range(QC) if only_tch is None else [only_tch]):
                qps = os_tile(tch % 4, "qps")
                for kc in range(CT):
                    nc.tensor.matmul(
                        qps[:], wq[:, kc, ts(g, 128)],
                        xT[:, kc, ts(tch, 512)],
                        start=(kc == 0), stop=(kc == CT - 1),
                    )
                nc.vector.tensor_scalar_add(
                    qT[:, g, ts(tch, 512)], qps[:], bq[:, g:g + 1])
                kps = os_tile(tch % 4, "kps")
                for kc in range(CT):
                    nc.tensor.matmul(
                        kps[:], wk[:, kc, ts(g, 128)],
                        xT[:, kc, ts(tch, 512)],
                        start=(kc == 0), stop=(kc == CT - 1),
                    )
                nc.vector.tensor_scalar_add(
                    kT[:, g, ts(tch, 512)], kps[:], bk[:, g:g + 1])

        def emit_v(xT, wv, tch=None):
            for tt in (range(TT) if tch is None else
                       range(4 * tch, 4 * tch + 4)):
                vps = os_tile(tt % 4, "vps")
                for kc in range(CT):
                    nc.tensor.matmul(
                        vps[:, 0:256], xT[:, kc, ts(tt, 128)], wv[:, kc, :],
                        start=(kc == 0), stop=(kc == CT - 1),
                    )
                nc.vector.tensor_copy(
                    vx[:, tt, :, 32:64],
                    vps[:, 0:256].rearrange("p (h d) -> p h d", h=HPC),
                )
                nc.vector.tensor_copy(vx[:, tt, :, 1:32], zero31[:])

        def emit_attention(g, after_qc=None):
            for c in range(QC):
                for h in range(4):
                    nc.sync.dma_start(k_st[h][:, ts(c, 512)],
                                      kT[ts(h, 32), g, ts(c, 512)])
            for qc in range(QC):
                qs = []
                for h in range(4):
                    qst = qstage.tile([32, 512], f32r, tag=f"qq{h}",
                                      name=f"qq{h}", bufs=2)
                    nc.sync.dma_start(qst[:], qT[ts(h, 32), g, ts(qc, 512)])
                    qs.append(qst)
                os_ps = [os_tile(h, f"os{h}") for h in range(4)]
                for kt in range(KT):
                    pT = pTp.tile([128, 4, 512], f32r, tag="pT", name="pT")
                    # two half-groups so exp(half A) overlaps the QK^T
                    # matmuls of half B on the PE
                    for half in range(2):
                        sT = psum.tile([128, 2, 512], f32, tag="sT", bufs=2,
                                       name="sT")
                        for i in range(2):
                            h = 2 * half + i
                            nc.tensor.matmul(
                                sT[:, i, :],
                                k_st[h][:, ts(kt, 128)],
                                qs[h][:, :],
                                start=True, stop=True,
                            )
                        nc.scalar.activation(
                            pT[:, 2 * half:2 * half + 2, :], sT[:],
                            Exp, scale=float(SCALE))
                    for h in range(4):
                        nc.tensor.matmul(
                            os_ps[h][0:64, :],
                            vx[:, kt, 4 * g + h, :],
                            pT[:, h, :],
                            start=(kt == 0), stop=(kt == KT - 1),
                        )
                for h in range(4):
                    r_sb = misc.tile([1, 512], f32, tag="r_sb", name="r_sb")
                    nc.vector.reciprocal(r_sb[:], os_ps[h][0:1, :])
                    rr = misc.tile([64, 512], f32, tag="rr", name="rr")
                    nc.gpsimd.partition_broadcast(rr[:], r_sb[:])
                    ot = misc.tile([64, 512], f32r, tag="ot", name="ot")
                    nc.vector.tensor_mul(ot[32:64, :], os_ps[h][32:64, :],
                                         rr[32:64, :])
                    nc.sync.dma_start(
                        oT_d.ap()[g, ts(h, 32), ts(qc, 512)], ot[32:64, :])
                if after_qc is not None:
                    after_qc(qc)

        def emit_proj(tch):
            o_stg = []
            for yt in range(2):
                og = ostage.tile([128, 512], f32r, tag=f"og{yt}",
                                 name=f"og{yt}", bufs=2)
                eng = nc.sync if yt == 0 else nc.scalar
                eng.dma_start(og[:], oT_d.ap()[yt, :, ts(tch, 512)])
                o_stg.append(og)
            for ct in range(CT):
                pps = os_tile(ct, "pps")
                for yt in range(2):
                    nc.tensor.matmul(
                        pps[:], wp[:, yt, ts(ct, 128)],
                        o_stg[yt][:, :],
                        start=(yt == 0), stop=(yt == 1),
                    )
                ost = ostage.tile([128, 512], f32, tag="ost", name="ost")
                nc.vector.tensor_copy(ost[:], pps[:])
                nc.scalar.dma_start(
                    outT_d.ap()[ts(ct, 128), ts(tch, 512)], ost[:])

        with tc.tile_pool(name="xTp", bufs=1) as xTp:
            wq = xTp.tile([128, CT, 256], f32r)
            wk = xTp.tile([128, CT, 256], f32r)
            wv = xTp.tile([128, CT, 256], f32r)
            emit_weights(wq, wk, wv)
            xT = xTp.tile([128, CT, T], f32r)
            anchor = None
            with tc.tile_pool(name="xstage", bufs=2) as xstage:
                for tch in range(QC):
                    emit_transposes(xT, xstage, tch)
                    emit_qk(xT, wq, wk, 0, tch)
                    emit_v(xT, wv, tch)
                    if tch == 0:
                        anchor = tc.tile_snap_priority()
            emit_qk(xT, wq, wk, 1)
        if "attn" in stages:
            with tc.high_priority(offset=tc.cur_priority - anchor):
                emit_attention(0)
            with tc.high_priority(offset=tc.cur_priority - anchor - 1000):
                emit_attention(1)
        if "proj" in stages:
            for tch in range(QC):
                emit_proj(tch)

    nc.compile()
    return nc


def _get_nc():
    if "nc" not in _CACHE:
        _CACHE["nc"] = _build()
    return _CACHE["nc"]


def kernel(x, w_attn, b_attn, w_proj, b_proj):
    from concourse.bass_utils import run_bass_kernel_spmd

    x = np.ascontiguousarray(np.asarray(x, dtype=np.float32))
    w_attn = np.asarray(w_attn, dtype=np.float32)
    b_attn = np.asarray(b_attn, dtype=np.float32)
    w_proj = np.ascontiguousarray(np.asarray(w_proj, dtype=np.float32))
    b_proj = np.asarray(b_proj, dtype=np.float32)

    nc = _get_nc()

    in_maps = []
    for core in range(N_CORES):
        b, hg = core // 2, core % 2
        cs = hg * 256  # head-column offset within each of q/k/v blocks
        wq = np.ascontiguousarray(w_attn[:, cs:cs + 256])
        wk = np.ascontiguousarray(w_attn[:, C + cs:C + cs + 256])
        wv = np.ascontiguousarray(w_attn[:, 2 * C + cs:2 * C + cs + 256])
        bq = np.ascontiguousarray(b_attn[cs:cs + 256].reshape(2, 128, 1))
        bk = np.ascontiguousarray(
            b_attn[C + cs:C + cs + 256].reshape(2, 128, 1))
        wp = np.ascontiguousarray(w_proj[cs:cs + 256, :])
        in_maps.append({
            "xb": np.ascontiguousarray(x[b]),
            "wq": wq, "wk": wk, "wv": wv, "bq": bq, "bk": bk, "wp": wp,
        })

    res = run_bass_kernel_spmd(nc, in_maps, core_ids=list(range(N_CORES)))

    b_eff = (b_proj + b_attn[2 * C:3 * C] @ w_proj).astype(np.float32)
    out = np.empty((B, T, C), dtype=np.float32)
    for b in range(B):
        acc = res.results[2 * b]["outT"].T + res.results[2 * b + 1]["outT"].T
        out[b] = acc + b_eff
    return out


if __name__ == "__main__":
    rng = np.random.default_rng(0)
    x = rng.standard_normal((B, T, C), dtype=np.float32)
    w_attn = (rng.standard_normal((C, 3 * C), dtype=np.float32) * 0.02)
    b_attn = (rng.standard_normal(3 * C, dtype=np.float32) * 0.02)
    w_proj = (rng.standard_normal((C, C), dtype=np.float32) * 0.02)
    b_proj = (rng.standard_normal(C, dtype=np.float32) * 0.02)
    out = kernel(x, w_attn, b_attn, w_proj, b_proj)
    print("kernel out", out.shape, out.dtype, float(np.abs(out).max()))
